# revision 50
# baseline (speedup 1.0000x reference)
"""Balanced CE loss + accuracy on 8 Trainium2 NeuronCores (Bass/Tile).

Reference computation (N = 16777216 elements):
    loss = -sum(where(t==1, 1.6*log(p), 0.4*log(1-p))) / N
    acc  = mean(round(p) == t)

Strategy (data-parallel over N, no collectives needed):
  Shard N across 8 cores; per core stream [128, C] chunks.

  Weight-in-the-log trick: w = 0.4+1.2t = 0.4*(1+3t), so
      sum w*ln(y) = 0.4 * sum ln(y^(1+3t)),   y = 1-|p-t| = |p+t-1|.
  One custom DVE op computes u = y^(1+3t) (= y if t==0 else y^4) in
  8 ALU slices:  c=1-t; d=p-c; y=max(d,c-p); u=min(y, y^4+c).
  y >= 1e-6 so u >= 1e-24 -- comfortably inside bf16 normal range.
  ONE ACT pass Ln(u) with fused accumulation then yields the whole
  per-chunk weighted log-sum; no second log pass, no cross term.

  A second custom DVE op counts correct predictions exactly in fp32:
      m = ((p-(1-t))^2 >= 0.25)  ==  (y >= 0.5)  ==  (round(p)==t),
  with fused accum -> per-chunk count (integers, exact in fp32).

  Engine budget per core (2M elems):
      DMA  ~41us  (16.8 MB at ~420 GB/s streaming)    <- bottleneck
      DVE  ~38.5us (2 passes at 1x: U-op + M-op)
      ACT  ~20us  (1 Ln pass + accumulator reads)
  All DVE reduce/accum/stt paths run at 1x regardless of dtype
  (measured: 2x/4x perf modes exist only for plain non-reduce ops), so
  the win over the 4-pass baseline comes from needing only 3 passes
  total and a shallow dependency graph: DMA -> {U,M} -> Ln.

  Pipeline facts baked into the shape below (from perfetto traces):
    - runtime preamble (engine barriers + TENSOR_LOAD + queue setup) is
      ~6.6us; the first DMA byte lands ~8.5us in.  Not controllable.
    - a chunk's compute is gated by its DMA-completion semaphore, which
      lags true data arrival by ~0.7x the chunk transfer time (16-engine
      straggler + event routing).  2048-col mid-chunks balance that lag
      against per-instruction overhead; finer sub-DMAs made it worse.
    - one-shot full-shard SBUF buffers (p 64KB + t 64KB per partition)
      eliminate all ring-recycle WAR waits.
    - tail chunks shrink (...,896,128) so the last data->U->Ln->accum
      chain is short, and all but the last 3 chunks' partials are
      DMA'd out early, overlapping the tail compute.  Finer tail ramps
      (6 small chunks) and coarser ones both measured worse.

  Host folds the [128, 2*NCH] partials in f64:
    loss = -0.4*sum(W)/N,  acc = sum(C)/N  (count is exact).
  Measured: ~59.6us typical / 59.2us best in clean HBM windows (vs
  67.9us baseline); cross-core HBM contention adds 0-12us rep noise.
  Rel err 4.2e-06 (count exact; loss error = bf16 u + Ln-table only).
"""

import sys

if "/opt/trn_rl_repo" not in sys.path:
    sys.path.insert(0, "/opt/trn_rl_repo")

import numpy as np

import concourse.bass as bass
import concourse.bacc as bacc
import concourse.tile as tile
from concourse import mybir
from concourse.bass_utils import run_bass_kernel_spmd
import concourse.hw_specs as hw_specs
import concourse.dve_ops as dve_ops
from concourse.dve_ops import DveOp, OPS, CUSTOM_DVE_SPECS
from concourse.dve_spec import (
    Spec, Src0, Src1, One, C0, sq, maxx, minn, lower, AluOp, _has_src1,
)
from concourse.dve_uop import DveOpSpec

N_CORES = 8
N = 16777216
P = 128
SHARD = N // N_CORES          # 2097152 elements per core
COLS = SHARD // P             # 16384 columns per core

# chunk sizes: small first chunks so compute starts early, shrinking tail
# chunks because the DMA-completion semaphore lags true data arrival by
# ~0.8x the chunk transfer time -- small tail chunks cut that lag where
# it matters (after the last byte)
SIZES = [512, 1024, 2048, 2048, 2048, 2048, 2048, 2048, 1536, 896, 128]
assert sum(SIZES) == COLS
NCH = len(SIZES)
MX = max(SIZES)
AF = mybir.ActivationFunctionType

_NC_CACHE = None

# Bias the Tile list-scheduler's DMA model slightly fast so it orders
# engine streams DMA-first (see baseline notes); harmless otherwise.
hw_specs.TRN2Spec.DMA_CYCLE = 1e9 / (400e9 / 128) / 1.05


def _ref_u(in0, in1, c0, c1, c2):
    t = in1.astype(np.float32)
    y = np.abs(in0.astype(np.float32) + t - 1.0)
    return np.minimum(y, np.square(np.square(y)) + (1.0 - t)).astype(np.float32)


def _ref_m(in0, in1, c0, c1, c2):
    t = in1.astype(np.float32)
    d = in0.astype(np.float32) + t - 1.0
    out = (d * d >= np.float32(c0)).astype(np.float32)
    acc = out.reshape(out.shape[0], -1).sum(axis=-1, keepdims=True)
    return out, acc


def _register_op(name, spec):
    if name in dve_ops._SUB_OPCODE_FOR_NAME:
        return next(op for op in OPS if op.name == name)
    row = max(dve_ops._SUB_OPCODE_FOR_NAME.values()) + 1
    assert row < 0x20
    dve_ops._SUB_OPCODE_FOR_NAME[name] = row
    shas = {}
    for ver in ("v3", "v4"):
        s = DveOpSpec(name=name, opcode=row, uops=lower(spec, ver=ver),
                      rd1_en=_has_src1(spec))
        shas[ver] = s.sha(ver)
    op = DveOp(name, spec, subdim=False, uops_sha=shas)
    OPS.append(op)
    CUSTOM_DVE_SPECS[name] = spec
    return op


def _register_custom_ops():
    # U: u = y^(1+3t),  y = |p+t-1|
    c = One - Src1
    d = Src0 - c
    y = maxx(d, c - Src0)
    u_body = minn(y, sq(sq(y)) + c)
    u_op = _register_op("U_WPOW_ANT", Spec(body=u_body, reference=_ref_u))
    # M: m = ((p-(1-t))^2 >= s0), accum add -> exact correct count
    m_body = sq(Src0 - (One - Src1)) >= C0
    m_op = _register_op(
        "M_COUNT_ANT", Spec(body=m_body, accum=AluOp.ADD, reference=_ref_m)
    )
    return u_op, m_op


def build_bass():
    """Build the single-core Bass program (SPMD across 8 cores)."""
    global _NC_CACHE
    if _NC_CACHE is not None:
        return _NC_CACHE

    u_op, m_op = _register_custom_ops()

    nc = bacc.Bacc("TRN2", target_bir_lowering=False, debug=False)

    p_in = nc.dram_tensor("p_in", [SHARD], mybir.dt.float32, kind="ExternalInput").ap()
    t_in = nc.dram_tensor("t_in", [SHARD], mybir.dt.int32, kind="ExternalInput").ap()
    # acc cols interleaved per chunk: [2s] = sum ln(u), [2s+1] = correct count
    acc = nc.dram_tensor("acc", [P, 2 * NCH], mybir.dt.float32, kind="ExternalOutput").ap()

    with tile.TileContext(nc) as tc:
        with (
            tc.tile_pool(name="misc", bufs=1) as misc_pool,
            tc.tile_pool(name="psj", bufs=1, space=bass.MemorySpace.PSUM) as psum_pool,
        ):
            warm = misc_pool.tile([P, 1], mybir.dt.float32, tag="warm")
            # per-chunk interleaved partials (col 2s = lnu-sum, 2s+1 = count)
            # so most columns can be DMA'd out early, overlapping the tail
            acc_t = misc_pool.tile([P, 2 * NCH], mybir.dt.float32, tag="acct")
            # one-shot full-shard input buffers: no ring recycling, so the
            # only DMA/compute gating is data arrival itself
            p_buf = misc_pool.tile([P, COLS], mybir.dt.float32, tag="pbuf")
            t_buf = misc_pool.tile([P, COLS], mybir.dt.int32, tag="tbuf")
            u_buf = misc_pool.tile([P, COLS], mybir.dt.bfloat16, tag="ubuf")
            junk_q = psum_pool.tile([P, MX], mybir.dt.float32, tag="jq")
            junk_m = psum_pool.tile([P, MX], mybir.dt.float32, tag="jm")

            offs = [sum(SIZES[:i]) for i in range(NCH)]

            def issue_dma(s):
                sz, o = SIZES[s], offs[s]
                # p on the Sync DGE ring, t on the ACT DGE ring: two queue
                # rings keep each tensor's descriptors contiguous, so a
                # chunk's completion semaphore isn't delayed by lookahead
                # descriptors of later chunks sharing the ring
                nc.sync.dma_start(
                    p_buf[:, o : o + sz],
                    p_in[o * P : (o + sz) * P].rearrange("(p f) -> p f", p=P),
                )
                nc.scalar.dma_start(
                    t_buf[:, o : o + sz],
                    t_in[o * P : (o + sz) * P].rearrange("(p f) -> p f", p=P),
                )

            def issue_compute(s):
                sz, o = SIZES[s], offs[s]
                p_t, t_t = p_buf[:, o : o + sz], t_buf[:, o : o + sz]
                u_t = u_buf[:, o : o + sz]
                # u = y^(1+3t)  (one fused DVE op)
                nc.vector._custom_dve(u_op, out=u_t, in0=p_t, in1=t_t)
                # exact correct-count with fused accum
                nc.vector._custom_dve(
                    m_op, out=junk_m[:, 0:sz], in0=p_t, in1=t_t, s0=0.25,
                    accum_out=acc_t[:, 2 * s + 1 : 2 * s + 2],
                )
                # weighted log-sum in one ACT pass: accum(ln u) = W_s/0.4
                nc.scalar.activation(
                    junk_q[:, 0:sz], u_t, AF.Ln, accum_out=acc_t[:, 2 * s : 2 * s + 1]
                )

            AHEAD = 2
            FLUSH_AT = NCH - 3  # chunks whose partials go in the early flush
            for s in range(NCH + AHEAD):
                if s < NCH:
                    issue_dma(s)
                if s == 1:
                    # warm the ACT Ln table off the critical path
                    nc.vector.memset(warm[:], 0.5)
                    nc.scalar.activation(warm[:], warm[:], AF.Ln)
                if s - AHEAD >= 0:
                    issue_compute(s - AHEAD)
                if s - AHEAD == FLUSH_AT - 1:
                    # early flush: completed chunks' partials overlap the tail
                    nc.sync.dma_start(
                        acc[:, 0 : 2 * FLUSH_AT], acc_t[:, 0 : 2 * FLUSH_AT]
                    )

            nc.sync.dma_start(
                acc[:, 2 * FLUSH_AT : 2 * NCH], acc_t[:, 2 * FLUSH_AT : 2 * NCH],
                single_packet=True,
            )

    nc.finalize()
    _NC_CACHE = nc
    return nc


def make_in_maps(input, target):
    inp = np.ascontiguousarray(np.asarray(input, dtype=np.float32)).reshape(
        N_CORES, SHARD
    )
    tgt = np.ascontiguousarray(np.asarray(target, dtype=np.int32)).reshape(
        N_CORES, SHARD
    )
    return [{"p_in": inp[c], "t_in": tgt[c]} for c in range(N_CORES)]


def combine(results, sum_t=None):
    """Host-side unshard: fold the 8 cores' partials -> (loss, acc)."""
    W = C = 0.0
    for r in results:
        aa = np.asarray(r["acc"], dtype=np.float64)
        W += aa[:, 0::2].sum()
        C += aa[:, 1::2].sum()
    loss = -0.4 * W / N
    acc = C / N
    return np.float32(loss), np.float32(acc)


def run_on_hw(input, target, **spmd_kwargs):
    nc = build_bass()
    in_maps = make_in_maps(input, target)
    return run_bass_kernel_spmd(nc, in_maps, list(range(N_CORES)), **spmd_kwargs)


def kernel(input, target):
    br = run_on_hw(input, target)
    return combine(br.results)
